# revision 24
# baseline (speedup 1.0000x reference)
"""Trainium2 Bass kernel for shifted sparse attention (nn_Attention_74672301408506).

Math (reference):
    q = x @ W.T ; k = x_key @ W.T ; att = softmax(q k^T)
    out[i, v] = sum_s w_s * sum_j att[i-2s, j] * x_value[j+2s, v]

Device algorithm (8 cores, query rows sharded, 512 rows per core, no halo):
    per core, query rows [r0, r0+512):
      zT[h, i] = (W^T W) x^T   (fp16 matmuls; G = W^T W folded on host)
      S^T[j, i] = x_keyT-tile^T @ zT   (scores transposed: keys on partitions)
      E = exp(S - 100)         (bf16; softmax is shift-invariant)
      Ru[i, :] = E^T @ [V_0|V_1|V_2|V_3|ones]   (ones col = softmax denominator)
      R = Ru[:, :320] / Ru[:, 320]
      C[t, v] = sum_s w_s R[t - 2s, 80s + v]  for t in [0, 518)  (banded matmuls)
    C rows [0, 512) are complete except the first 6 rows, which miss the
    neighbor's contribution; rows [512, 518) are exactly that contribution for
    the next core. The host adds the 6-row overlaps when unsharding (exact).

All HBM streams are host-pre-tiled so every DMA moves >=2KB per partition
line (HWDGE ring throughput collapses with small descriptor lines).
"""

import os
import sys
import types

import numpy as np
import ml_dtypes

T = 4096
Q = 256
H = 256
NV = 80
NS = 4
STEP = 2
NCORES = 8
M = T // NCORES            # 512 rows per core
CSUB = 100.0               # global score shift before exp
P = 128
NJ = T // P                # 32 key tiles
NF = Q // P                # 2 feature tiles
NH = H // P                # 2 hidden tiles
NMAIN = M // P             # 4 i-chunks of 128
NVC = NS * NV + 1          # value width incl ones column
TAIL = (NS - 1) * STEP     # 6 overlap rows between neighboring cores
NWARM = 7                  # big PE warmup matmuls (HAM clock ramp)
NWSM = 8                   # small trailing warmup matmuls (64-wide)
XKB = (4, 4, 8, 8, 8)      # xk group DMA sizes (j-tiles), all on sync ring
VCB = (4, 4, 8, 16)        # vcomb DMA block sizes (j-tiles), all on gpsimd
AUXW = 8 * P + NS * TAIL   # aux width (appended to the vcomb stream)
OW = (NMAIN + 1) * NV      # output width: 4 chunks + tail-rows column block


def _install_axon_ntff_hook():
    """bass_utils' trace path imports antenv.axon_hooks, which the agent image
    lacks; shim it and register the ctypes-based NTFF profiler hook."""
    if "antenv.axon_hooks" in sys.modules:
        return
    try:
        import antenv
    except ImportError:
        return
    mod = types.ModuleType("antenv.axon_hooks")
    mod._hook = None
    mod.set_axon_ntff_profile_hook = lambda h: setattr(mod, "_hook", h)
    mod.get_axon_ntff_profile_hook = lambda: mod._hook
    sys.modules["antenv.axon_hooks"] = mod
    antenv.axon_hooks = mod
    try:
        from trn_agent_boot import trn_boot

        so_path = "/opt/axon/libaxon_pjrt.so"
        if os.path.exists(so_path):
            mod.set_axon_ntff_profile_hook(trn_boot._ntff_profile_via_ctypes(so_path))
    except Exception:
        pass


_NC_CACHE = {}
LAST_RESULT = None


def _build_nc():
    import concourse.mybir as mybir
    import concourse.tile as tile
    from concourse import bacc

    f32 = mybir.dt.float32
    f16 = mybir.dt.float16
    bf16 = mybir.dt.bfloat16
    Exp = mybir.ActivationFunctionType.Exp

    nc = bacc.Bacc(None, target_bir_lowering=False)

    # packed inputs (see _host_prep for layouts)
    ztp_d = nc.dram_tensor("ztp", [P, NF * M], f16, kind="ExternalInput")
    xkg_d = nc.dram_tensor("xkg", [P, NJ * NF * P], f16, kind="ExternalInput")
    vc_d = nc.dram_tensor("vcomb", [P, NJ * NVC + AUXW], bf16, kind="ExternalInput")
    out_d = nc.dram_tensor("out", [P, OW], f16, kind="ExternalOutput")

    with tile.TileContext(nc) as tc:
        with (
            tc.tile_pool(name="consts", bufs=1) as consts,
            tc.tile_pool(name="io", bufs=1) as io,
            tc.tile_pool(name="store", bufs=1) as store,
            tc.tile_pool(name="small", bufs=6) as small,
            tc.tile_pool(name="psA", bufs=4, space="PSUM") as psA,
            tc.tile_pool(name="psR", bufs=4, space="PSUM") as psR,
        ):
            # ---- PE warmup: dummy matmuls while input DMAs stream, so HAM
            # reaches K=8/8 (2.4 GHz) before real work ----
            wu = consts.tile([P, 512], bf16, name="wu")
            nc.vector.memset(wu, 0.0)
            wups = psA.tile([P, 512], f32, name="wups", tag="ps")
            for i in range(NWARM):
                nc.tensor.matmul(wups, wu[:, 0:P], wu, start=True, stop=True)
            # small trailing warmups: keep the PE continuously busy (HAM stays
            # ramped) until the first real inputs land, with tiny overshoot
            for i in range(NWSM):
                nc.tensor.matmul(
                    wups[:, 0:64], wu[:, 0:P], wu[:, 0:64], start=True, stop=True
                )

            # ---- input DMAs (10 total; fewer DMAs = fewer semaphores = less
            # end-of-program bookkeeping). The two HWDGE rings (sync+scalar)
            # share one descriptor frontend, so the xk stream lives entirely
            # on sync in consumption order (no priority inversion); scalar
            # issues only ztp (its engine must reach the exp stream quickly);
            # gpsimd (SWDGE) streams vcomb blocks with aux appended.
            xkall = io.tile([P, NJ * NF * P], f16, name="xkall", tag="xkall")
            W1 = NF * P

            def load_xk(j0, nb, eng):
                eng.dma_start(
                    out=xkall[:, W1 * j0 : W1 * (j0 + nb)],
                    in_=xkg_d[:, W1 * j0 : W1 * (j0 + nb)],
                )

            ztall = consts.tile([P, NF * M], f16, name="ztall", tag="ztall")
            nc.scalar.dma_start(out=ztall, in_=ztp_d[:, :])
            j0 = 0
            for nb in XKB:
                load_xk(j0, nb, nc.sync)
                j0 += nb

            vcall = store.tile([P, NJ * NVC + AUXW], bf16, name="vcall", tag="vc")
            j0 = 0
            for bi, nb in enumerate(VCB):
                w1 = NVC * (j0 + nb) + (AUXW if bi == len(VCB) - 1 else 0)
                nc.gpsimd.dma_start(
                    out=vcall[:, NVC * j0 : w1], in_=vc_d[:, NVC * j0 : w1]
                )
                j0 += nb
            aux = vcall[:, NJ * NVC : NJ * NVC + AUXW]

            vc = [vcall[:, NVC * j : NVC * (j + 1)] for j in range(NJ)]
            bias_t = consts.tile([P, 1], f32, name="bias_t")
            nc.vector.memset(bias_t, -CSUB)

            zt = [ztall[:, f * M : (f + 1) * M] for f in range(NF)]

            def xk_piece(f, j):
                return xkall[:, (j * NF + f) * P : (j * NF + f + 1) * P]

            # ---- pipeline: S^T(j) = x_keyT-tile^T @ zT; exp; Ru lagging
            # DELAY j-tiles ----
            ru = []
            for c in range(NMAIN):
                ru.append(psR.tile([P, NVC], f32, name=f"ru{c}", tag="ru"))
            elist = []
            DELAY = 3

            def ru_step(j):
                for c in range(NMAIN):
                    nc.tensor.matmul(
                        ru[c],
                        elist[j][:, P * c : P * (c + 1)],
                        vc[j],
                        start=(j == 0),
                        stop=(j == NJ - 1),
                    )

            for j in range(NJ):
                ps = psA.tile([P, M], f32, name=f"sps{j}", tag="ps")
                for f in range(NF):
                    nc.tensor.matmul(
                        ps,
                        xk_piece(f, j),
                        zt[f],
                        start=(f == 0),
                        stop=(f == NF - 1),
                    )
                ej = store.tile([P, M], bf16, name=f"e{j}", tag="E", bufs=NJ)
                nc.scalar.activation(ej, ps, Exp, bias=bias_t)
                elist.append(ej)
                if j >= DELAY:
                    ru_step(j - DELAY)
            for j in range(NJ - DELAY, NJ):
                ru_step(j)

            # ---- normalize + combine, chunk by chunk ----
            # aux layout (bf16), all [128, 128] banded matrices with w_s baked:
            #   A1 = aux[:, 128s:128(s+1)]        k == p - 2s       (own chunk)
            #   A2 = aux[:, 512+128s:512+128(s+1)] k == 128 + p - 2s (prev chunk)
            #   A3 = aux[:, 1024+6s:1024+6(s+1)]  k == 128 + t' - 2s (tail rows)
            rch = []
            for c in range(NMAIN):
                rec = small.tile([P, 1], f32, name=f"rec{c}", tag="rec")
                nc.vector.reciprocal(rec, ru[c][:, NS * NV : NVC])
                t = store.tile([P, NS * NV], bf16, name=f"r{c}", tag=f"r{c}")
                # normalize on the Act engine (per-partition scale operand)
                nc.scalar.mul(t, ru[c][:, 0 : NS * NV], rec)
                rch.append(t)

            oall = small.tile([P, OW], f16, name="oall", tag="osb")

            def combine_chunk(c):
                po = psA.tile([P, NV], f32, name=f"po{c}", tag="ps")
                nmm = NS + (NS - 1 if c > 0 else 0)
                i = 0
                for s in range(NS):
                    nc.tensor.matmul(
                        po,
                        aux[:, P * s : P * (s + 1)],
                        rch[c][:, NV * s : NV * (s + 1)],
                        start=(i == 0),
                        stop=(i == nmm - 1),
                    )
                    i += 1
                if c > 0:
                    for s in range(1, NS):
                        nc.tensor.matmul(
                            po,
                            aux[:, 4 * P + P * s : 4 * P + P * (s + 1)],
                            rch[c - 1][:, NV * s : NV * (s + 1)],
                            start=False,
                            stop=(i == nmm - 1),
                        )
                        i += 1
                if c % 2 == 0:
                    nc.vector.tensor_copy(oall[:, NV * c : NV * (c + 1)], po)
                else:
                    nc.scalar.copy(oall[:, NV * c : NV * (c + 1)], po)

            for c in range(NMAIN - 1):
                combine_chunk(c)
            # tail rows [512, 518) (next core's missing contribution) go into
            # the extra column block; computed before chunk 3 so its copy
            # overlaps chunk 3's matmuls
            pot = psA.tile([TAIL, NV], f32, name="pot", tag="ps")
            for s in range(1, NS):
                nc.tensor.matmul(
                    pot,
                    aux[:, 8 * P + TAIL * s : 8 * P + TAIL * (s + 1)],
                    rch[NMAIN - 1][:, NV * s : NV * (s + 1)],
                    start=(s == 1),
                    stop=(s == NS - 1),
                )
            nc.scalar.copy(oall[0:TAIL, NMAIN * NV : OW], pot)
            combine_chunk(NMAIN - 1)
            nc.sync.dma_start(out=out_d[:, :], in_=oall)

    nc.compile()
    return nc


def _get_nc():
    if "nc" not in _NC_CACHE:
        _install_axon_ntff_hook()
        _NC_CACHE["nc"] = _build_nc()
    return _NC_CACHE["nc"]


def _host_prep(x, x_key, x_value, W_qk, w_shift):
    bf = ml_dtypes.bfloat16
    x = np.ascontiguousarray(np.asarray(x, dtype=np.float32))
    x_key = np.ascontiguousarray(np.asarray(x_key, dtype=np.float32))
    x_value = np.ascontiguousarray(np.asarray(x_value, dtype=np.float32))
    W_qk = np.ascontiguousarray(np.asarray(W_qk, dtype=np.float32))
    w_shift = np.asarray(w_shift, dtype=np.float32)

    xkT = x_key.T.astype(np.float16)                         # [Q, T]
    # pre-tile: [NF, P, NJ, P] -> [P, NJ, NF, P]: partition line p holds, for
    # each j-tile, [f0 row p, f1 row p] (2KB per 4-tile group DMA)
    xkg = np.ascontiguousarray(
        xkT.reshape(NF, P, NJ, P).transpose(1, 2, 0, 3).reshape(P, NJ * NF * P)
    )
    gmat = (W_qk.astype(np.float64).T @ W_qk.astype(np.float64)).astype(np.float32)

    vcomb = np.zeros((T, NVC), np.float32)
    for s in range(NS):
        d = STEP * s
        vcomb[: T - d, NV * s : NV * (s + 1)] = x_value[d:, :]
    vcomb[:, NS * NV] = 1.0
    # pre-tile: [T, NVC] -> [NJ, P, NVC] -> [P, NJ*NVC] so each SBUF partition
    # line is one contiguous DMA descriptor
    vcomb = (
        vcomb.astype(bf).reshape(NJ, P, NVC).transpose(1, 0, 2).reshape(P, NJ * NVC)
    )

    # combine matrices (see aux layout comment in _build_nc)
    aux = np.zeros((P, 8 * P + NS * TAIL), np.float32)
    for s in range(NS):
        w = w_shift[0, s]
        for p in range(P):
            k = p - STEP * s
            if 0 <= k < P:
                aux[k, P * s + p] = w                      # A1
            kk = P + p - STEP * s
            if 0 <= kk < P:
                aux[kk, 4 * P + P * s + p] = w             # A2 (prev chunk)
        if s >= 1:
            for tp in range(TAIL):
                k = P + tp - STEP * s
                if 0 <= k < P:
                    aux[k, 8 * P + TAIL * s + tp] = w      # A3 (tail rows)
    aux = aux.astype(bf)
    vcomb = np.ascontiguousarray(np.concatenate([vcomb, aux], axis=1))

    # z projection on host (same scale as the G=W^T W fold): z = G x^T, then
    # pack as [P, NF*M]: line p = [z[p, :], z[128+p, :]]
    zall = (gmat @ x.T).astype(np.float16)                   # [Q, T]
    in_maps = []
    for d in range(NCORES):
        r0 = d * M
        zc = zall[:, r0 : r0 + M]                            # [Q, M]
        ztp = np.ascontiguousarray(
            np.concatenate([zc[:P], zc[P:]], axis=1, dtype=np.float16)
        )
        in_maps.append({"ztp": ztp, "xkg": xkg, "vcomb": vcomb})
    return in_maps


def kernel(x, x_key, x_value, W_qk, w_shift):
    global LAST_RESULT
    from concourse.bass_utils import run_bass_kernel_spmd

    nc = _get_nc()
    in_maps = _host_prep(x, x_key, x_value, W_qk, w_shift)
    res = run_bass_kernel_spmd(nc, in_maps, core_ids=list(range(NCORES)))
    LAST_RESULT = res
    outs = [np.asarray(res.results[d]["out"], dtype=np.float32) for d in range(NCORES)]
    out = np.concatenate(
        [
            o[:, : NMAIN * NV].reshape(P, NMAIN, NV).transpose(1, 0, 2).reshape(M, NV)
            for o in outs
        ],
        axis=0,
    )
    # add the 6-row cross-core overlap contributions (tail column block)
    for d in range(NCORES - 1):
        out[M * (d + 1) : M * (d + 1) + TAIL] += outs[d][0:TAIL, NMAIN * NV : OW]
    return out.astype(np.float32)


# revision 25
# speedup vs baseline: 1.1840x; 1.1840x over previous
"""Trainium2 Bass kernel for shifted sparse attention (nn_Attention_74672301408506).

Math (reference):
    q = x @ W.T ; k = x_key @ W.T ; att = softmax(q k^T)
    out[i, v] = sum_s w_s * sum_j att[i-2s, j] * x_value[j+2s, v]

Device algorithm (8 cores, query rows sharded, 512 rows per core, no halo):
    per core, query rows [r0, r0+512):
      zT[h, i] = (W^T W) x^T   (fp16 matmuls; G = W^T W folded on host)
      S^T[j, i] = x_keyT-tile^T @ zT   (scores transposed: keys on partitions)
      E = exp(S - 100)         (bf16; softmax is shift-invariant)
      Ru[i, :] = E^T @ [V_0|V_1|V_2|V_3|ones]   (ones col = softmax denominator)
      R = Ru[:, :320] / Ru[:, 320]
      C[t, v] = sum_s w_s R[t - 2s, 80s + v]  for t in [0, 518)  (banded matmuls)
    C rows [0, 512) are complete except the first 6 rows, which miss the
    neighbor's contribution; rows [512, 518) are exactly that contribution for
    the next core. The host adds the 6-row overlaps when unsharding (exact).

All HBM streams are host-pre-tiled so every DMA moves >=2KB per partition
line (HWDGE ring throughput collapses with small descriptor lines).
"""

import os
import sys
import types

import numpy as np
import ml_dtypes

T = 4096
Q = 256
H = 256
NV = 80
NS = 4
STEP = 2
NCORES = 8
M = T // NCORES            # 512 rows per core
CSUB = 100.0               # global score shift before exp
P = 128
NJ = T // P                # 32 key tiles
NF = Q // P                # 2 feature tiles
NH = H // P                # 2 hidden tiles
NMAIN = M // P             # 4 i-chunks of 128
NVC = NS * NV + 1          # value width incl ones column
TAIL = (NS - 1) * STEP     # 6 overlap rows between neighboring cores
NWARM = 9                  # big PE warmup matmuls (HAM clock ramp)
NWSM = 0                   # small trailing warmup matmuls (64-wide)
XKB = (4, 4, 8, 8, 8)      # xk group DMA sizes (j-tiles), all on sync ring
VCB = (4, 4, 8, 16)        # vcomb DMA block sizes (j-tiles), all on gpsimd
AUXW = 8 * P + NS * TAIL   # aux width (appended to the vcomb stream)
OW = (NMAIN + 1) * NV      # output width: 4 chunks + tail-rows column block


def _install_axon_ntff_hook():
    """bass_utils' trace path imports antenv.axon_hooks, which the agent image
    lacks; shim it and register the ctypes-based NTFF profiler hook."""
    if "antenv.axon_hooks" in sys.modules:
        return
    try:
        import antenv
    except ImportError:
        return
    mod = types.ModuleType("antenv.axon_hooks")
    mod._hook = None
    mod.set_axon_ntff_profile_hook = lambda h: setattr(mod, "_hook", h)
    mod.get_axon_ntff_profile_hook = lambda: mod._hook
    sys.modules["antenv.axon_hooks"] = mod
    antenv.axon_hooks = mod
    try:
        from trn_agent_boot import trn_boot

        so_path = "/opt/axon/libaxon_pjrt.so"
        if os.path.exists(so_path):
            mod.set_axon_ntff_profile_hook(trn_boot._ntff_profile_via_ctypes(so_path))
    except Exception:
        pass


_NC_CACHE = {}
LAST_RESULT = None


def _build_nc():
    import concourse.mybir as mybir
    import concourse.tile as tile
    from concourse import bacc

    f32 = mybir.dt.float32
    f16 = mybir.dt.float16
    bf16 = mybir.dt.bfloat16
    Exp = mybir.ActivationFunctionType.Exp

    nc = bacc.Bacc(None, target_bir_lowering=False)

    # packed inputs (see _host_prep for layouts)
    ztp_d = nc.dram_tensor("ztp", [P, NF * M], f16, kind="ExternalInput")
    xkg_d = nc.dram_tensor("xkg", [P, NJ * NF * P], f16, kind="ExternalInput")
    vc_d = nc.dram_tensor("vcomb", [P, NJ * NVC + AUXW], bf16, kind="ExternalInput")
    out_d = nc.dram_tensor("out", [P, OW], f16, kind="ExternalOutput")

    with tile.TileContext(nc) as tc:
        with (
            tc.tile_pool(name="consts", bufs=1) as consts,
            tc.tile_pool(name="io", bufs=1) as io,
            tc.tile_pool(name="store", bufs=1) as store,
            tc.tile_pool(name="small", bufs=6) as small,
            tc.tile_pool(name="psA", bufs=4, space="PSUM") as psA,
            tc.tile_pool(name="psR", bufs=4, space="PSUM") as psR,
        ):
            # ---- PE warmup: dummy matmuls while input DMAs stream, so HAM
            # reaches K=8/8 (2.4 GHz) before real work ----
            wu = consts.tile([P, 512], bf16, name="wu")
            nc.vector.memset(wu, 0.0)
            wups = psA.tile([P, 512], f32, name="wups", tag="ps")
            for i in range(NWARM):
                nc.tensor.matmul(wups, wu[:, 0:P], wu, start=True, stop=True)
            # small trailing warmups: keep the PE continuously busy (HAM stays
            # ramped) until the first real inputs land, with tiny overshoot
            for i in range(NWSM):
                nc.tensor.matmul(
                    wups[:, 0:64], wu[:, 0:P], wu[:, 0:64], start=True, stop=True
                )

            # ---- input DMAs (10 total; fewer DMAs = fewer semaphores = less
            # end-of-program bookkeeping). The two HWDGE rings (sync+scalar)
            # share one descriptor frontend, so the xk stream lives entirely
            # on sync in consumption order (no priority inversion); scalar
            # issues only ztp (its engine must reach the exp stream quickly);
            # gpsimd (SWDGE) streams vcomb blocks with aux appended.
            xkall = io.tile([P, NJ * NF * P], f16, name="xkall", tag="xkall")
            W1 = NF * P

            def load_xk(j0, nb, eng):
                eng.dma_start(
                    out=xkall[:, W1 * j0 : W1 * (j0 + nb)],
                    in_=xkg_d[:, W1 * j0 : W1 * (j0 + nb)],
                )

            ztall = consts.tile([P, NF * M], f16, name="ztall", tag="ztall")
            nc.scalar.dma_start(out=ztall, in_=ztp_d[:, :])
            j0 = 0
            for nb in XKB:
                load_xk(j0, nb, nc.sync)
                j0 += nb

            vcall = store.tile([P, NJ * NVC + AUXW], bf16, name="vcall", tag="vc")
            j0 = 0
            for bi, nb in enumerate(VCB):
                w1 = NVC * (j0 + nb) + (AUXW if bi == len(VCB) - 1 else 0)
                nc.gpsimd.dma_start(
                    out=vcall[:, NVC * j0 : w1], in_=vc_d[:, NVC * j0 : w1]
                )
                j0 += nb
            aux = vcall[:, NJ * NVC : NJ * NVC + AUXW]

            vc = [vcall[:, NVC * j : NVC * (j + 1)] for j in range(NJ)]
            bias_t = consts.tile([P, 1], f32, name="bias_t")
            nc.vector.memset(bias_t, -CSUB)

            zt = [ztall[:, f * M : (f + 1) * M] for f in range(NF)]

            def xk_piece(f, j):
                return xkall[:, (j * NF + f) * P : (j * NF + f + 1) * P]

            # ---- pipeline: S^T(j) = x_keyT-tile^T @ zT; exp; Ru lagging
            # DELAY j-tiles ----
            ru = []
            for c in range(NMAIN):
                ru.append(psR.tile([P, NVC], f32, name=f"ru{c}", tag="ru"))
            elist = []
            DELAY = 2

            def ru_step(j):
                for c in range(NMAIN):
                    nc.tensor.matmul(
                        ru[c],
                        elist[j][:, P * c : P * (c + 1)],
                        vc[j],
                        start=(j == 0),
                        stop=(j == NJ - 1),
                    )

            for j in range(NJ):
                ps = psA.tile([P, M], f32, name=f"sps{j}", tag="ps")
                for f in range(NF):
                    nc.tensor.matmul(
                        ps,
                        xk_piece(f, j),
                        zt[f],
                        start=(f == 0),
                        stop=(f == NF - 1),
                    )
                ej = store.tile([P, M], bf16, name=f"e{j}", tag="E", bufs=NJ)
                nc.scalar.activation(ej, ps, Exp, bias=bias_t)
                elist.append(ej)
                if j >= DELAY:
                    ru_step(j - DELAY)
            for j in range(NJ - DELAY, NJ):
                ru_step(j)

            # ---- normalize + combine, chunk by chunk ----
            # aux layout (bf16), all [128, 128] banded matrices with w_s baked:
            #   A1 = aux[:, 128s:128(s+1)]        k == p - 2s       (own chunk)
            #   A2 = aux[:, 512+128s:512+128(s+1)] k == 128 + p - 2s (prev chunk)
            #   A3 = aux[:, 1024+6s:1024+6(s+1)]  k == 128 + t' - 2s (tail rows)
            rch = []
            for c in range(NMAIN):
                rec = small.tile([P, 1], f32, name=f"rec{c}", tag="rec")
                nc.vector.reciprocal(rec, ru[c][:, NS * NV : NVC])
                t = store.tile([P, NS * NV], bf16, name=f"r{c}", tag=f"r{c}")
                # normalize on the Act engine (per-partition scale operand)
                nc.scalar.mul(t, ru[c][:, 0 : NS * NV], rec)
                rch.append(t)

            oall = small.tile([P, OW], f16, name="oall", tag="osb")

            def combine_chunk(c):
                po = psA.tile([P, NV], f32, name=f"po{c}", tag="ps")
                nmm = NS + (NS - 1 if c > 0 else 0)
                i = 0
                for s in range(NS):
                    nc.tensor.matmul(
                        po,
                        aux[:, P * s : P * (s + 1)],
                        rch[c][:, NV * s : NV * (s + 1)],
                        start=(i == 0),
                        stop=(i == nmm - 1),
                    )
                    i += 1
                if c > 0:
                    for s in range(1, NS):
                        nc.tensor.matmul(
                            po,
                            aux[:, 4 * P + P * s : 4 * P + P * (s + 1)],
                            rch[c - 1][:, NV * s : NV * (s + 1)],
                            start=False,
                            stop=(i == nmm - 1),
                        )
                        i += 1
                if c % 2 == 0:
                    nc.vector.tensor_copy(oall[:, NV * c : NV * (c + 1)], po)
                else:
                    nc.scalar.copy(oall[:, NV * c : NV * (c + 1)], po)

            for c in range(NMAIN - 1):
                combine_chunk(c)
            # tail rows [512, 518) (next core's missing contribution) go into
            # the extra column block; computed before chunk 3 so its copy
            # overlaps chunk 3's matmuls
            pot = psA.tile([TAIL, NV], f32, name="pot", tag="ps")
            for s in range(1, NS):
                nc.tensor.matmul(
                    pot,
                    aux[:, 8 * P + TAIL * s : 8 * P + TAIL * (s + 1)],
                    rch[NMAIN - 1][:, NV * s : NV * (s + 1)],
                    start=(s == 1),
                    stop=(s == NS - 1),
                )
            nc.scalar.copy(oall[0:TAIL, NMAIN * NV : OW], pot)
            combine_chunk(NMAIN - 1)
            nc.sync.dma_start(out=out_d[:, :], in_=oall)

    nc.compile()
    return nc


def _get_nc():
    if "nc" not in _NC_CACHE:
        _install_axon_ntff_hook()
        _NC_CACHE["nc"] = _build_nc()
    return _NC_CACHE["nc"]


def _host_prep(x, x_key, x_value, W_qk, w_shift):
    bf = ml_dtypes.bfloat16
    x = np.ascontiguousarray(np.asarray(x, dtype=np.float32))
    x_key = np.ascontiguousarray(np.asarray(x_key, dtype=np.float32))
    x_value = np.ascontiguousarray(np.asarray(x_value, dtype=np.float32))
    W_qk = np.ascontiguousarray(np.asarray(W_qk, dtype=np.float32))
    w_shift = np.asarray(w_shift, dtype=np.float32)

    xkT = x_key.T.astype(np.float16)                         # [Q, T]
    # pre-tile: [NF, P, NJ, P] -> [P, NJ, NF, P]: partition line p holds, for
    # each j-tile, [f0 row p, f1 row p] (2KB per 4-tile group DMA)
    xkg = np.ascontiguousarray(
        xkT.reshape(NF, P, NJ, P).transpose(1, 2, 0, 3).reshape(P, NJ * NF * P)
    )
    gmat = (W_qk.astype(np.float64).T @ W_qk.astype(np.float64)).astype(np.float32)

    vcomb = np.zeros((T, NVC), np.float32)
    for s in range(NS):
        d = STEP * s
        vcomb[: T - d, NV * s : NV * (s + 1)] = x_value[d:, :]
    vcomb[:, NS * NV] = 1.0
    # pre-tile: [T, NVC] -> [NJ, P, NVC] -> [P, NJ*NVC] so each SBUF partition
    # line is one contiguous DMA descriptor
    vcomb = (
        vcomb.astype(bf).reshape(NJ, P, NVC).transpose(1, 0, 2).reshape(P, NJ * NVC)
    )

    # combine matrices (see aux layout comment in _build_nc)
    aux = np.zeros((P, 8 * P + NS * TAIL), np.float32)
    for s in range(NS):
        w = w_shift[0, s]
        for p in range(P):
            k = p - STEP * s
            if 0 <= k < P:
                aux[k, P * s + p] = w                      # A1
            kk = P + p - STEP * s
            if 0 <= kk < P:
                aux[kk, 4 * P + P * s + p] = w             # A2 (prev chunk)
        if s >= 1:
            for tp in range(TAIL):
                k = P + tp - STEP * s
                if 0 <= k < P:
                    aux[k, 8 * P + TAIL * s + tp] = w      # A3 (tail rows)
    aux = aux.astype(bf)
    vcomb = np.ascontiguousarray(np.concatenate([vcomb, aux], axis=1))

    # z projection on host (same scale as the G=W^T W fold): z = G x^T, then
    # pack as [P, NF*M]: line p = [z[p, :], z[128+p, :]]
    zall = (gmat @ x.T).astype(np.float16)                   # [Q, T]
    in_maps = []
    for d in range(NCORES):
        r0 = d * M
        zc = zall[:, r0 : r0 + M]                            # [Q, M]
        ztp = np.ascontiguousarray(
            np.concatenate([zc[:P], zc[P:]], axis=1, dtype=np.float16)
        )
        in_maps.append({"ztp": ztp, "xkg": xkg, "vcomb": vcomb})
    return in_maps


def kernel(x, x_key, x_value, W_qk, w_shift):
    global LAST_RESULT
    from concourse.bass_utils import run_bass_kernel_spmd

    nc = _get_nc()
    in_maps = _host_prep(x, x_key, x_value, W_qk, w_shift)
    res = run_bass_kernel_spmd(nc, in_maps, core_ids=list(range(NCORES)))
    LAST_RESULT = res
    outs = [np.asarray(res.results[d]["out"], dtype=np.float32) for d in range(NCORES)]
    out = np.concatenate(
        [
            o[:, : NMAIN * NV].reshape(P, NMAIN, NV).transpose(1, 0, 2).reshape(M, NV)
            for o in outs
        ],
        axis=0,
    )
    # add the 6-row cross-core overlap contributions (tail column block)
    for d in range(NCORES - 1):
        out[M * (d + 1) : M * (d + 1) + TAIL] += outs[d][0:TAIL, NMAIN * NV : OW]
    return out.astype(np.float32)
